# revision 5
# baseline (speedup 1.0000x reference)
"""RWKV-v4 block (time-mix + channel-mix) on 8 Trainium2 NeuronCores.

Sequence-parallel: core i owns tokens [i*256, (i+1)*256) of both batches;
weights replicated, streamed bf16 from HBM. The WKV recurrence runs in
unstabilized linear form A' = lam*A + e^k v (safe: lam = e^{-exp(td)} < 1,
positive bounded terms) via the DVE tensor_tensor_scan primitive with
channels on partitions. Cross-core carries via one small AllGather of
per-core summaries (weighted sums of e^k terms), combined on-device with
host-precomputed per-channel decay powers. Activations channel-major
[c, (b, t)]; LN stats via PE ones-matmuls; division via exp(-ln(x)) on
ScalarE; all large matmuls bf16 with fp32 PSUM accumulation.
"""
import contextlib

import numpy as np
import ml_dtypes

import concourse.bass as bass
import concourse.mybir as mybir
import concourse.tile as tile
from concourse.bass_utils import run_bass_kernel_spmd

P = 128
B, T, C, F = 2, 2048, 2048, 8192
W = 8
Tl = T // W      # 256 local output tokens per batch
Ta = Tl + 1      # 257: attention range (incl. boundary token t0-1)
Tx = Ta + 1      # 258: ln1 range (incl. t0-2)
CB = C // P      # 16
FO = F // P      # 64
LN_EPS = 1e-5

f32 = mybir.dt.float32
bf16 = mybir.dt.bfloat16
AF = mybir.ActivationFunctionType
OP = mybir.AluOpType

(I_TMK, I_TMV, I_TMR, I_FMK, I_FMR, I_G1, I_B1, I_G2, I_B2, I_EU,
 I_INVL, I_BMASK, I_NLAM, I_BMINV, I_PAD0, I_PAD1) = range(16)

_MAXW = 1  # this walrus build allows only 1 sync wait per instruction


def _fix_sync_waits(nc):
    for f in nc.m.functions:
        for bb in f.blocks:
            new, changed = [], False
            for inst in bb.instructions:
                si = inst.sync_info
                if si is not None and si.on_wait and len(si.on_wait) > _MAXW:
                    waits = list(si.on_wait)
                    extra, keep = waits[:-_MAXW], waits[-_MAXW:]
                    for ci in range(0, len(extra), _MAXW):
                        nop = mybir.InstNoOp(
                            name=f"{inst.name}-wfix{ci}", engine=inst.engine
                        )
                        nop.sync_info = mybir.SyncInfo(
                            on_wait=extra[ci : ci + _MAXW], on_update=[]
                        )
                        new.append(nop)
                    inst.sync_info = mybir.SyncInfo(
                        on_wait=keep, on_update=list(si.on_update)
                    )
                    changed = True
                new.append(inst)
            if changed:
                bb.instructions = new


def build_program():
    nc = bass.Bass("TRN2", target_bir_lowering=False)

    xin = nc.dram_tensor("xin", [P, CB, B, Tx], f32, kind="ExternalInput")
    wk = nc.dram_tensor("wk", [CB, C, P], bf16, kind="ExternalInput")
    wv = nc.dram_tensor("wv", [CB, C, P], bf16, kind="ExternalInput")
    wr = nc.dram_tensor("wr", [CB, C, P], bf16, kind="ExternalInput")
    wo = nc.dram_tensor("wo", [CB, C, P], bf16, kind="ExternalInput")
    fr = nc.dram_tensor("fr", [CB, C, P], bf16, kind="ExternalInput")
    fk = nc.dram_tensor("fk", [FO, C, P], bf16, kind="ExternalInput")
    fv = nc.dram_tensor("fv", [CB, F, P], bf16, kind="ExternalInput")
    lam_bc = nc.dram_tensor("lam_bc", [P, CB, Ta], f32, kind="ExternalInput")
    lpow = nc.dram_tensor("lpow", [P, CB, Ta], f32, kind="ExternalInput")
    lrev = nc.dram_tensor("lrev", [P, CB, Ta], f32, kind="ExternalInput")
    pc = nc.dram_tensor("pc", [P, 16, CB], f32, kind="ExternalInput")
    dmat = nc.dram_tensor("dmat", [P, W, CB], f32, kind="ExternalInput")
    end_da = nc.dram_tensor("end_da", [P, W, CB], f32, kind="ExternalInput")
    end_po = nc.dram_tensor("end_po", [P, W, CB], f32, kind="ExternalInput")
    carry0 = nc.dram_tensor("carry0", [P, CB, 2, B], f32, kind="ExternalInput")
    st0end = nc.dram_tensor("st0end", [P, CB, 3, B], f32, kind="ExternalInput")
    attsh = nc.dram_tensor("attsh", [P, CB, B], f32, kind="ExternalInput")
    ffnsh = nc.dram_tensor("ffnsh", [P, CB, B], f32, kind="ExternalInput")

    o_x = nc.dram_tensor("o_x", [CB, P, B, Tl], f32, kind="ExternalOutput")
    o_x1l = nc.dram_tensor("o_x1l", [P, CB, B], f32, kind="ExternalOutput")
    o_x2l = nc.dram_tensor("o_x2l", [P, CB, B], f32, kind="ExternalOutput")
    o_wkv = nc.dram_tensor("o_wkv", [P, CB, 3, B], f32, kind="ExternalOutput")

    cc_in = nc.dram_tensor("cc_in", [P, 3 * CB * B], f32)
    cc_out = nc.dram_tensor("cc_out", [W, P, 3 * CB * B], f32,
                            addr_space="Shared")
    adram = nc.dram_tensor("adram", [P, CB, B, Ta], f32)

    with tile.TileContext(nc) as tc, contextlib.ExitStack() as top:
        perm = top.enter_context(tc.tile_pool(name="perm", bufs=1))
        trans = top.enter_context(tc.tile_pool(name="trans", bufs=1))
        wpool = top.enter_context(tc.tile_pool(name="wpool", bufs=1))
        psmm = top.enter_context(tc.tile_pool(name="psmm", bufs=1, space="PSUM"))
        psst = top.enter_context(tc.tile_pool(name="psst", bufs=1, space="PSUM"))

        def tb(name):
            return trans.tile([P, B, Tx], f32, name=name, tag="tb", bufs=8)

        def ty(name):
            return trans.tile([P, Ta], f32, name=name, tag="ty", bufs=6)

        def wt(name):
            return wpool.tile([P, CB, P], bf16, name=name, tag="wt", bufs=4)

        def psa(name):
            return psmm.tile([P, B, 512], f32, name=name, tag="mm", bufs=2)

        # ---- small persistent constants/state ----
        pc_t = perm.tile([P, 16, CB], f32, name="pc_t")
        nc.sync.dma_start(pc_t[:], pc[:])
        dmat_t = perm.tile([P, W, CB], f32, name="dmat_t")
        nc.sync.dma_start(dmat_t[:], dmat[:])
        endda_t = perm.tile([P, W, CB], f32, name="endda_t")
        nc.sync.dma_start(endda_t[:], end_da[:])
        endpo_t = perm.tile([P, W, CB], f32, name="endpo_t")
        nc.sync.dma_start(endpo_t[:], end_po[:])
        carry0_t = perm.tile([P, CB, 2, B], f32, name="carry0_t")
        nc.sync.dma_start(carry0_t[:], carry0[:])
        st0_t = perm.tile([P, CB, 3, B], f32, name="st0_t")
        nc.sync.dma_start(st0_t[:], st0end[:])
        attsh_t = perm.tile([P, CB, B], f32, name="attsh_t")
        nc.sync.dma_start(attsh_t[:], attsh[:])
        ffnsh_t = perm.tile([P, CB, B], f32, name="ffnsh_t")
        nc.sync.dma_start(ffnsh_t[:], ffnsh[:])
        ones_t = perm.tile([P, P], f32, name="ones_t")
        nc.vector.memset(ones_t[:], 1.0)
        eps_t = perm.tile([P, 1], f32, name="eps_t")
        nc.vector.memset(eps_t[:], LN_EPS)
        x1l_t = perm.tile([P, CB, B], f32, name="x1l_t")
        x2l_t = perm.tile([P, CB, B], f32, name="x2l_t")
        pack_t = perm.tile([P, 3, CB, B], f32, name="pack_t")
        gath_t = perm.tile([P, W, 3, CB, B], f32, name="gath_t")
        accA = perm.tile([P, CB, B], f32, name="accA")
        accB = perm.tile([P, CB, B], f32, name="accB")
        tmpc = perm.tile([P, CB, B], f32, name="tmpc")
        cA1 = perm.tile([P, CB, B], f32, name="cA1")
        cB1 = perm.tile([P, CB, B], f32, name="cB1")
        wkvst = perm.tile([P, CB, 3, B], f32, name="wkvst")
        enp = perm.tile([P, CB, B], f32, name="enp")
        m1_t = perm.tile([P, B, Tx], f32, name="m1_t")
        rs1_t = perm.tile([P, B, Tx], f32, name="rs1_t")
        m2_t = perm.tile([P, B, Ta], f32, name="m2_t")
        rs2_t = perm.tile([P, B, Ta], f32, name="rs2_t")

        def pcv(i, cb):
            return pc_t[:, i, cb : cb + 1]

        with contextlib.ExitStack() as attctx:
            pEV = attctx.enter_context(tc.tile_pool(name="pEV", bufs=1))
            ek_t = pEV.tile([P, CB, B, Ta], bf16, name="ek_t")
            v_t = pEV.tile([P, CB, B, Ta], bf16, name="v_t")
            xr_t = pEV.tile([P, CB, B, Ta], bf16, name="xr_t")

            # ============ phase A: ln1, mixes, k/v matmuls, summaries =====
            with contextlib.ExitStack() as actx:
                pKV = actx.enter_context(tc.tile_pool(name="pKV", bufs=1))
                lrev_t = pKV.tile([P, CB, Ta], f32, name="lrev_t")
                nc.sync.dma_start(lrev_t[:], lrev[:])
                xk_t = pKV.tile([P, CB, B, Ta], bf16, name="xk_t")
                xv_t = pKV.tile([P, CB, B, Ta], bf16, name="xv_t")

                with contextlib.ExitStack() as lctx:
                    pLN = lctx.enter_context(tc.tile_pool(name="pLN", bufs=1))
                    x_t = pLN.tile([P, CB, B, Tx], f32, name="x_t")
                    nc.sync.dma_start(x_t[:], xin[:])

                    ps_st = psst.tile(
                        [P, 2, B, 512], f32, name="ps_st", tag="ps_st"
                    )
                    for cb in range(CB):
                        xsq = tb("xsq")
                        nc.scalar.activation(xsq[:], x_t[:, cb], AF.Square)
                        for b in range(B):
                            nc.tensor.matmul(
                                ps_st[:, 0, b, :Tx], ones_t[:], x_t[:, cb, b],
                                start=(cb == 0), stop=(cb == CB - 1),
                            )
                            nc.tensor.matmul(
                                ps_st[:, 1, b, :Tx], ones_t[:], xsq[:, b],
                                start=(cb == 0), stop=(cb == CB - 1),
                            )
                    nc.vector.tensor_scalar(
                        m1_t[:], ps_st[:, 0, :, :Tx], 1.0 / C, 0.0,
                        OP.mult, OP.add,
                    )
                    msq = tb("msq")
                    nc.vector.tensor_tensor(
                        msq[:, :, :Tx], m1_t[:], m1_t[:], OP.mult
                    )
                    nc.vector.scalar_tensor_tensor(
                        rs1_t[:], ps_st[:, 1, :, :Tx], 1.0 / C, msq[:, :, :Tx],
                        OP.mult, OP.subtract,
                    )
                    nc.scalar.activation(rs1_t[:], rs1_t[:], AF.Ln, bias=eps_t[:, 0:1])
                    nc.scalar.activation(rs1_t[:], rs1_t[:], AF.Exp, scale=-0.5)

                    for cb in range(CB):
                        x1 = tb("x1")
                        nc.vector.tensor_tensor(
                            x1[:], x_t[:, cb], m1_t[:], OP.subtract
                        )
                        nc.vector.tensor_tensor(
                            x1[:], x1[:], rs1_t[:], OP.mult
                        )
                        nc.vector.tensor_scalar(
                            x1[:], x1[:], pcv(I_G1, cb), pcv(I_B1, cb),
                            OP.mult, OP.add,
                        )
                        for b in range(B):
                            nc.vector.tensor_scalar_mul(
                                x1[:, b, 1:2], x1[:, b, 1:2], pcv(I_BMASK, cb)
                            )
                            nc.vector.scalar_tensor_tensor(
                                x1[:, b, 1:2], attsh_t[:, cb, b : b + 1],
                                pcv(I_BMINV, cb), x1[:, b, 1:2],
                                OP.mult, OP.add,
                            )
                        nc.vector.tensor_copy(x1l_t[:, cb], x1[:, :, Tx - 1])
                        d = tb("d")
                        nc.vector.tensor_tensor(
                            d[:, :, :Ta], x1[:, :, 1:Tx], x1[:, :, 0:Ta],
                            OP.subtract,
                        )
                        for b in range(B):
                            nc.vector.scalar_tensor_tensor(
                                xk_t[:, cb, b], d[:, b, :Ta], pcv(I_TMK, cb),
                                x1[:, b, 0:Ta], OP.mult, OP.add,
                            )
                            nc.vector.scalar_tensor_tensor(
                                xv_t[:, cb, b], d[:, b, :Ta], pcv(I_TMV, cb),
                                x1[:, b, 0:Ta], OP.mult, OP.add,
                            )
                            nc.vector.scalar_tensor_tensor(
                                xr_t[:, cb, b], d[:, b, :Ta], pcv(I_TMR, cb),
                                x1[:, b, 0:Ta], OP.mult, OP.add,
                            )

                # ---- k/v projections + summaries per c_out block ----
                for cb in range(CB):
                    wkt = wt("wkt")
                    nc.sync.dma_start(
                        wkt[:], wk[cb].rearrange("(k p) m -> p k m", p=P)
                    )
                    wvt = wt("wvt")
                    nc.sync.dma_start(
                        wvt[:], wv[cb].rearrange("(k p) m -> p k m", p=P)
                    )
                    ps_k = psa("ps_k")
                    for k in range(CB):
                        for b in range(B):
                            nc.tensor.matmul(
                                ps_k[:, b, :Ta], wkt[:, k], xk_t[:, k, b],
                                start=(k == 0), stop=(k == CB - 1),
                            )
                    ps_v = psa("ps_v")
                    for k in range(CB):
                        for b in range(B):
                            nc.tensor.matmul(
                                ps_v[:, b, :Ta], wvt[:, k], xv_t[:, k, b],
                                start=(k == 0), stop=(k == CB - 1),
                            )
                    ekf = tb("ekf")
                    nc.scalar.activation(
                        ekf[:, :, :Ta], ps_k[:, :, :Ta], AF.Exp
                    )
                    nc.vector.tensor_scalar_mul(
                        ekf[:, :, 0:1], ekf[:, :, 0:1], pcv(I_BMASK, cb)
                    )
                    nc.vector.tensor_copy(ek_t[:, cb], ekf[:, :, :Ta])
                    nc.scalar.activation(
                        v_t[:, cb], ps_v[:, :, :Ta], AF.Copy
                    )
                    # summaries: tmp = ekf * lrev ; S_B = sum, m = ln(max)
                    #            tv = tmp * v ; S_A = sum
                    for b in range(B):
                        tmp = ty("tmp")
                        nc.vector.tensor_tensor(
                            tmp[:], ekf[:, b, :Ta], lrev_t[:, cb], OP.mult
                        )
                        nc.vector.tensor_reduce(
                            pack_t[:, 1, cb, b : b + 1], tmp[:],
                            mybir.AxisListType.X, OP.add,
                        )
                        nc.vector.tensor_reduce(
                            pack_t[:, 2, cb, b : b + 1], tmp[:],
                            mybir.AxisListType.X, OP.max,
                        )
                        tv = ty("tv")
                        nc.vector.tensor_tensor(
                            tv[:], tmp[:], ps_v[:, b, :Ta], OP.mult
                        )
                        nc.vector.tensor_reduce(
                            pack_t[:, 0, cb, b : b + 1], tv[:],
                            mybir.AxisListType.X, OP.add,
                        )
                # m = ln(max(tmp)) ; harmless on A/B rows (overwritten? no):
                nc.scalar.activation(
                    pack_t[:, 2], pack_t[:, 2], AF.Ln
                )

                # ---- collective ----
                nc.sync.dma_start(
                    cc_in[:], pack_t.rearrange("p a c b -> p (a c b)")
                )
                nc.gpsimd.collective_compute(
                    "AllGather", OP.bypass,
                    replica_groups=[list(range(W))],
                    ins=[cc_in[:]], outs=[cc_out[:]],
                )
                nc.sync.dma_start(
                    gath_t[:],
                    cc_out.rearrange("w p q -> p w q").rearrange(
                        "p w (a c b) -> p w a c b", a=3, c=CB
                    ),
                )

            # ============ phase B: r matmuls, carries, y ===================
            with contextlib.ExitStack() as bctx:
                pB = bctx.enter_context(tc.tile_pool(name="pB", bufs=1))
                lam_t = pB.tile([P, CB, Ta], f32, name="lam_t")
                nc.sync.dma_start(lam_t[:], lam_bc[:])
                lpow_t = pB.tile([P, CB, Ta], f32, name="lpow_t")
                nc.sync.dma_start(lpow_t[:], lpow[:])
                sr_t = pB.tile([P, CB, B, Ta], bf16, name="sr_t")
                yo_t = pB.tile([P, CB, B, Ta], bf16, name="yo_t")

                # r projections + sigmoid via tanh (overlaps collective)
                for cb in range(CB):
                    wrt = wt("wrt")
                    nc.sync.dma_start(
                        wrt[:], wr[cb].rearrange("(k p) m -> p k m", p=P)
                    )
                    ps_r = psa("ps_r")
                    for k in range(CB):
                        for b in range(B):
                            nc.tensor.matmul(
                                ps_r[:, b, :Ta], wrt[:, k], xr_t[:, k, b],
                                start=(k == 0), stop=(k == CB - 1),
                            )
                    th = tb("th")
                    nc.scalar.activation(
                        th[:, :, :Ta], ps_r[:, :, :Ta], AF.Tanh, scale=0.5
                    )
                    nc.vector.tensor_scalar(
                        sr_t[:, cb], th[:, :, :Ta], 0.5, 0.5, OP.mult, OP.add
                    )

                # carries from gathered summaries
                nc.vector.tensor_copy(accA[:], carry0_t[:, :, 0])
                nc.vector.tensor_copy(accB[:], carry0_t[:, :, 1])
                for j in range(W):
                    dj = dmat_t[:, j, :, None].to_broadcast((P, CB, B))
                    nc.vector.tensor_tensor(
                        tmpc[:], gath_t[:, j, 0], dj, OP.mult
                    )
                    nc.vector.tensor_tensor(accA[:], accA[:], tmpc[:], OP.add)
                    nc.vector.tensor_tensor(
                        tmpc[:], gath_t[:, j, 1], dj, OP.mult
                    )
                    nc.vector.tensor_tensor(accB[:], accB[:], tmpc[:], OP.add)
                # back one token: c1 = (acc - beta0) * inv_lam
                b0 = perm.tile([P, CB, B], f32, name="b0")
                nc.vector.tensor_tensor(
                    b0[:], ek_t[:, :, :, 0], v_t[:, :, :, 0], OP.mult
                )
                nc.vector.tensor_tensor(cA1[:], accA[:], b0[:], OP.subtract)
                nc.vector.tensor_tensor(
                    cB1[:], accB[:], ek_t[:, :, :, 0], OP.subtract
                )
                for cb in range(CB):
                    nc.vector.tensor_scalar_mul(
                        cA1[:, cb], cA1[:, cb], pcv(I_INVL, cb)
                    )
                    nc.vector.tensor_scalar_mul(
                        cB1[:, cb], cB1[:, cb], pcv(I_INVL, cb)
                    )

                # final wkv state (identical on every core)
                for s in range(3):
                    nc.vector.tensor_copy(wkvst[:, :, s], st0_t[:, :, s])
                for j in range(W):
                    dj = endda_t[:, j, :, None].to_broadcast((P, CB, B))
                    nc.vector.tensor_tensor(
                        tmpc[:], gath_t[:, j, 0], dj, OP.mult
                    )
                    nc.vector.tensor_tensor(
                        wkvst[:, :, 0], wkvst[:, :, 0], tmpc[:], OP.add
                    )
                    nc.vector.tensor_tensor(
                        tmpc[:], gath_t[:, j, 1], dj, OP.mult
                    )
                    nc.vector.tensor_tensor(
                        wkvst[:, :, 1], wkvst[:, :, 1], tmpc[:], OP.add
                    )
                    pj = endpo_t[:, j, :, None].to_broadcast((P, CB, B))
                    nc.vector.tensor_tensor(
                        tmpc[:], gath_t[:, j, 2], pj, OP.add
                    )
                    nc.vector.tensor_tensor(
                        wkvst[:, :, 2], wkvst[:, :, 2], tmpc[:], OP.max
                    )
                nc.scalar.activation(
                    enp[:], wkvst[:, :, 2], AF.Exp, scale=-1.0
                )
                nc.vector.tensor_tensor(
                    wkvst[:, :, 0], wkvst[:, :, 0], enp[:], OP.mult
                )
                nc.vector.tensor_tensor(
                    wkvst[:, :, 1], wkvst[:, :, 1], enp[:], OP.mult
                )
                nc.sync.dma_start(o_wkv[:], wkvst[:])

                # ---- y per (cb, b): scans + carry-adjust + divide ----
                for cb in range(CB):
                    for b in range(B):
                        ekv = ty("ekv")
                        nc.vector.tensor_tensor(
                            ekv[:], ek_t[:, cb, b], v_t[:, cb, b], OP.mult
                        )
                        sca = trans.tile(
                            [P, Ta + 1], f32, name="sca", tag="sc", bufs=4
                        )
                        nc.vector.memset(sca[:, 0:1], 0.0)
                        nc.vector.tensor_tensor_scan(
                            sca[:, 1:], lam_t[:, cb], ekv[:],
                            0.0, OP.mult, OP.add,
                        )
                        scb = trans.tile(
                            [P, Ta + 1], f32, name="scb", tag="sc", bufs=4
                        )
                        nc.vector.memset(scb[:, 0:1], 0.0)
                        nc.vector.tensor_tensor_scan(
                            scb[:, 1:], lam_t[:, cb], ek_t[:, cb, b],
                            0.0, OP.mult, OP.add,
                        )
                        num = ty("num")
                        nc.vector.scalar_tensor_tensor(
                            num[:], lpow_t[:, cb], cA1[:, cb, b : b + 1],
                            sca[:, 0:Ta], OP.mult, OP.add,
                        )
                        nc.vector.scalar_tensor_tensor(
                            num[:], ekv[:], pcv(I_EU, cb), num[:],
                            OP.mult, OP.add,
                        )
                        den = ty("den")
                        nc.vector.scalar_tensor_tensor(
                            den[:], lpow_t[:, cb], cB1[:, cb, b : b + 1],
                            scb[:, 0:Ta], OP.mult, OP.add,
                        )
                        nc.vector.scalar_tensor_tensor(
                            den[:], ek_t[:, cb, b], pcv(I_EU, cb), den[:],
                            OP.mult, OP.add,
                        )
                        nc.scalar.activation(den[:], den[:], AF.Ln)
                        nc.scalar.activation(den[:], den[:], AF.Exp, scale=-1.0)
                        nc.vector.tensor_tensor(num[:], num[:], den[:], OP.mult)
                        nc.vector.tensor_tensor(
                            yo_t[:, cb, b], num[:], sr_t[:, cb, b], OP.mult
                        )

                # ---- Wo + residual + ln2 stats, attn -> DRAM ----
                ps_st2 = psst.tile(
                    [P, 2, B, 512], f32, name="ps_st2", tag="ps_st"
                )
                for co in range(CB):
                    wot = wt("wot")
                    nc.sync.dma_start(
                        wot[:], wo[co].rearrange("(k p) m -> p k m", p=P)
                    )
                    ps_o = psa("ps_o")
                    for k in range(CB):
                        for b in range(B):
                            nc.tensor.matmul(
                                ps_o[:, b, :Ta], wot[:, k], yo_t[:, k, b],
                                start=(k == 0), stop=(k == CB - 1),
                            )
                    xres = tb("xres")
                    nc.sync.dma_start(
                        xres[:, :, :Ta], xin[:, co, :, 1:Tx]
                    )
                    at = tb("at")
                    nc.vector.tensor_tensor(
                        at[:, :, :Ta], ps_o[:, :, :Ta], xres[:, :, :Ta], OP.add
                    )
                    nc.sync.dma_start(adram[:, co], at[:, :, :Ta])
                    asq = tb("asq")
                    nc.scalar.activation(
                        asq[:, :, :Ta], at[:, :, :Ta], AF.Square
                    )
                    for b in range(B):
                        nc.tensor.matmul(
                            ps_st2[:, 0, b, :Ta], ones_t[:], at[:, b, :Ta],
                            start=(co == 0), stop=(co == CB - 1),
                        )
                        nc.tensor.matmul(
                            ps_st2[:, 1, b, :Ta], ones_t[:], asq[:, b, :Ta],
                            start=(co == 0), stop=(co == CB - 1),
                        )
                nc.vector.tensor_scalar(
                    m2_t[:], ps_st2[:, 0, :, :Ta], 1.0 / C, 0.0, OP.mult, OP.add
                )
                msq2 = tb("msq2")
                nc.vector.tensor_tensor(
                    msq2[:, :, :Ta], m2_t[:], m2_t[:], OP.mult
                )
                nc.vector.scalar_tensor_tensor(
                    rs2_t[:], ps_st2[:, 1, :, :Ta], 1.0 / C, msq2[:, :, :Ta],
                    OP.mult, OP.subtract,
                )
                nc.scalar.activation(rs2_t[:], rs2_t[:], AF.Ln, bias=eps_t[:, 0:1])
                nc.scalar.activation(rs2_t[:], rs2_t[:], AF.Exp, scale=-0.5)

        # ============ FFN =================================================
        with contextlib.ExitStack() as fctx:
            pF = fctx.enter_context(tc.tile_pool(name="pF", bufs=1))
            xk2_t = pF.tile([P, CB, B, Tl], bf16, name="xk2_t")
            xr2_t = pF.tile([P, CB, B, Tl], bf16, name="xr2_t")
            sig_t = pF.tile([P, CB, B, Tl], bf16, name="sig_t")

            for cb in range(CB):
                a2 = tb("a2")
                nc.sync.dma_start(a2[:, :, :Ta], adram[:, cb])
                x2 = tb("x2")
                nc.vector.tensor_tensor(
                    x2[:, :, :Ta], a2[:, :, :Ta], m2_t[:], OP.subtract
                )
                nc.vector.tensor_tensor(
                    x2[:, :, :Ta], x2[:, :, :Ta], rs2_t[:], OP.mult
                )
                nc.vector.tensor_scalar(
                    x2[:, :, :Ta], x2[:, :, :Ta], pcv(I_G2, cb),
                    pcv(I_B2, cb), OP.mult, OP.add,
                )
                for b in range(B):
                    nc.vector.tensor_scalar_mul(
                        x2[:, b, 0:1], x2[:, b, 0:1], pcv(I_BMASK, cb)
                    )
                    nc.vector.scalar_tensor_tensor(
                        x2[:, b, 0:1], ffnsh_t[:, cb, b : b + 1],
                        pcv(I_BMINV, cb), x2[:, b, 0:1], OP.mult, OP.add,
                    )
                nc.vector.tensor_copy(x2l_t[:, cb], x2[:, :, Ta - 1])
                d2 = tb("d2")
                nc.vector.tensor_tensor(
                    d2[:, :, :Tl], x2[:, :, 1:Ta], x2[:, :, 0:Tl], OP.subtract
                )
                for b in range(B):
                    nc.vector.scalar_tensor_tensor(
                        xk2_t[:, cb, b], d2[:, b, :Tl], pcv(I_FMK, cb),
                        x2[:, b, 0:Tl], OP.mult, OP.add,
                    )
                    nc.vector.scalar_tensor_tensor(
                        xr2_t[:, cb, b], d2[:, b, :Tl], pcv(I_FMR, cb),
                        x2[:, b, 0:Tl], OP.mult, OP.add,
                    )

            for co in range(CB):
                frt = wt("frt")
                nc.sync.dma_start(
                    frt[:], fr[co].rearrange("(k p) m -> p k m", p=P)
                )
                ps_fr = psa("ps_fr")
                for k in range(CB):
                    for b in range(B):
                        nc.tensor.matmul(
                            ps_fr[:, b, :Tl], frt[:, k], xr2_t[:, k, b],
                            start=(k == 0), stop=(k == CB - 1),
                        )
                nc.scalar.activation(sig_t[:, co], ps_fr[:, :, :Tl], AF.Sigmoid)

            kf_t = pF.tile([P, FO, B, Tl], bf16, name="kf_t")
            for fo in range(FO):
                fkt = wt("fkt")
                nc.sync.dma_start(
                    fkt[:], fk[fo].rearrange("(k p) m -> p k m", p=P)
                )
                ps_fk = psa("ps_fk")
                for k in range(CB):
                    for b in range(B):
                        nc.tensor.matmul(
                            ps_fk[:, b, :Tl], fkt[:, k], xk2_t[:, k, b],
                            start=(k == 0), stop=(k == CB - 1),
                        )
                rl = tb("rl")
                nc.scalar.activation(rl[:, :, :Tl], ps_fk[:, :, :Tl], AF.Relu)
                nc.vector.tensor_tensor(
                    kf_t[:, fo], rl[:, :, :Tl], rl[:, :, :Tl], OP.mult
                )

            for co in range(CB):
                fvt = pF.tile(
                    [P, FO, P], bf16, name="fvt", tag="fvt", bufs=2
                )
                nc.sync.dma_start(
                    fvt[:], fv[co].rearrange("(k p) m -> p k m", p=P)
                )
                ps_fv = psa("ps_fv")
                for k in range(FO):
                    for b in range(B):
                        nc.tensor.matmul(
                            ps_fv[:, b, :Tl], fvt[:, k], kf_t[:, k, b],
                            start=(k == 0), stop=(k == FO - 1),
                        )
                ares = tb("ares")
                nc.sync.dma_start(ares[:, :, :Ta], adram[:, co])
                outt = tb("outt")
                for b in range(B):
                    nc.vector.tensor_tensor(
                        outt[:, b, :Tl], sig_t[:, co, b], ps_fv[:, b, :Tl],
                        OP.mult,
                    )
                    nc.vector.tensor_tensor(
                        outt[:, b, :Tl], outt[:, b, :Tl],
                        ares[:, b, 1:Ta], OP.add,
                    )
                nc.sync.dma_start(o_x[co], outt[:, :, :Tl])

        nc.sync.dma_start(o_x1l[:], x1l_t[:])
        nc.sync.dma_start(o_x2l[:], x2l_t[:])

    _fix_sync_waits(nc)
    return nc


# ---------------------------------------------------------------------------
# host-side preparation
# ---------------------------------------------------------------------------

_cache = {}


def _chan(v):  # [C] -> [P, CB]
    return np.ascontiguousarray(
        np.asarray(v, np.float32).reshape(CB, P).transpose(1, 0)
    )


def _prep_static(td, tf, weights, vecs):
    (Wk, Wv, Wr, Wo, Fk, Fr, Fv) = weights
    (tmk, tmv, tmr, fmk, fmr, g1, b1, g2, b2) = vecs
    w = -np.exp(np.asarray(td, np.float64))
    lam = np.exp(w)
    eu = np.exp(np.asarray(tf, np.float64))

    pc = np.zeros((P, 16, CB), np.float32)
    for i, v in enumerate([tmk, tmv, tmr, fmk, fmr, g1, b1, g2, b2, eu]):
        pc[:, i] = _chan(v)
    pc[:, I_NLAM] = _chan(-lam)

    s = np.arange(Ta, dtype=np.float64)
    lam_bc_h = np.broadcast_to(lam[:, None], (C, Ta))
    lpow_h = np.exp(w[:, None] * s[None, :])
    lrev_h = np.exp(w[:, None] * (Tl - s)[None, :])
    lrev_h = lrev_h.copy()
    lrev_h[:, 0] = 0.0

    def slots(a):
        return np.ascontiguousarray(
            np.asarray(a, np.float32).reshape(CB, P, Ta).transpose(1, 0, 2)
        )

    def wcb(a):  # [W, C] -> [P, W, CB]
        return np.ascontiguousarray(
            np.asarray(a, np.float32).reshape(W, CB, P).transpose(2, 0, 1)
        )

    end_da = np.stack([np.exp(w * (Tl * (W - 1 - j))) for j in range(W)])
    end_po = np.stack([Tl * (W - 1 - j) * w for j in range(W)])
    dmats = []
    for i in range(W):
        dm = np.zeros((W, C), np.float64)
        for j in range(i):
            dm[j] = np.exp(w * (Tl * (i - 1 - j)))
        dmats.append(wcb(dm))

    def wmat(Wm, no):  # [K, M] -> [no, K, P] bf16
        K = Wm.shape[0]
        return np.ascontiguousarray(
            np.asarray(Wm).astype(ml_dtypes.bfloat16)
            .reshape(K, no, P).transpose(1, 0, 2)
        )

    static = dict(
        wk=wmat(Wk, CB), wv=wmat(Wv, CB), wr=wmat(Wr, CB), wo=wmat(Wo, CB),
        fr=wmat(Fr, CB), fk=wmat(Fk, FO), fv=wmat(Fv, CB),
        lam_bc=slots(lam_bc_h), lpow=slots(lpow_h), lrev=slots(lrev_h),
        pc=pc, end_da=wcb(end_da), end_po=wcb(end_po),
    )
    return static, dmats, w, lam


def kernel(x, att_shift, wkv_state, ffn_shift, ln1_g, ln1_b, ln2_g, ln2_b,
           tmk, tmv, tmr, time_decay, time_first, Wk, Wv, Wr, Wo,
           fmk, fmr, Fk, Fr, Fv, _trace=False):
    if "nc" not in _cache:
        _cache["nc"] = build_program()
    nc = _cache["nc"]
    key = (id(Wk), id(Fk))
    if _cache.get("wkey") != key:
        _cache["static"], _cache["dmats"], _cache["w"], _cache["lam"] = (
            _prep_static(
                time_decay, time_first,
                tuple(np.asarray(a) for a in (Wk, Wv, Wr, Wo, Fk, Fr, Fv)),
                tuple(np.asarray(a) for a in
                      (tmk, tmv, tmr, fmk, fmr, ln1_g, ln1_b, ln2_g, ln2_b)),
            )
        )
        _cache["wkey"] = key
    static, dmats, w, lam = (
        _cache["static"], _cache["dmats"], _cache["w"], _cache["lam"]
    )

    x = np.asarray(x, np.float32)
    wkv_state = np.asarray(wkv_state, np.float64)
    aa0, bb0, pp0 = wkv_state[:, 0], wkv_state[:, 1], wkv_state[:, 2]
    ppc = np.clip(pp0, -1e30, 80.0)
    A0 = aa0 * np.exp(ppc)
    B0 = bb0 * np.exp(ppc)

    def cbp(a):  # [B, C] -> [P, CB, B]
        return np.ascontiguousarray(
            np.asarray(a, np.float32).reshape(B, CB, P).transpose(2, 1, 0)
        )

    x_cbt = np.ascontiguousarray(x.transpose(2, 0, 1))  # [C, B, T]
    attsh_h = cbp(att_shift)
    ffnsh_h = cbp(ffn_shift)

    in_maps = []
    for i in range(W):
        t0 = i * Tl
        xin = np.zeros((C, B, Tx), np.float32)
        lo = max(0, t0 - 2)
        xin[:, :, lo - (t0 - 2):] = x_cbt[:, :, lo : t0 + Tl]
        xin = np.ascontiguousarray(
            xin.reshape(CB, P, B, Tx).transpose(1, 0, 2, 3)
        )
        c0 = np.zeros((P, CB, 2, B), np.float32)
        s0 = np.zeros((P, CB, 3, B), np.float32)
        for b in range(B):
            c0[:, :, 0, b] = _chan(A0[b] * np.exp(w * t0))
            c0[:, :, 1, b] = _chan(B0[b] * np.exp(w * t0))
            s0[:, :, 0, b] = _chan(A0[b] * np.exp(w * T))
            s0[:, :, 1, b] = _chan(B0[b] * np.exp(w * T))
            s0[:, :, 2, b] = _chan(np.maximum(ppc[b] + w * T, -1e30))
        pc = static["pc"].copy()
        if i == 0:
            pc[:, I_INVL] = 1.0
            pc[:, I_BMASK] = 0.0
            pc[:, I_BMINV] = 1.0
        else:
            pc[:, I_INVL] = _chan(1.0 / lam)
            pc[:, I_BMASK] = 1.0
            pc[:, I_BMINV] = 0.0
        m = dict(static)
        m["pc"] = pc
        m["xin"] = xin
        m["dmat"] = dmats[i]
        m["carry0"] = c0
        m["st0end"] = s0
        m["attsh"] = attsh_h
        m["ffnsh"] = ffnsh_h
        in_maps.append(m)

    res = run_bass_kernel_spmd(
        nc, in_maps, core_ids=list(range(W)), trace=_trace
    )

    xs = []
    for i in range(W):
        o = res.results[i]["o_x"]  # [CB, P, B, Tl]
        xs.append(o.transpose(2, 3, 0, 1).reshape(B, Tl, C))
    x_out = np.concatenate(xs, axis=1)
    r7 = res.results[W - 1]
    x1l = r7["o_x1l"].transpose(2, 1, 0).reshape(B, C)
    x2l = r7["o_x2l"].transpose(2, 1, 0).reshape(B, C)
    wkv = r7["o_wkv"].transpose(3, 2, 1, 0).reshape(B, 3, C)
    if _trace:
        kernel.last_exec_ns = res.exec_time_ns
    return x_out, x1l, wkv, x2l
